# revision 3
# baseline (speedup 1.0000x reference)
"""Trainium2 Bass kernel for nn_Interpolator: zero-stuff upsample x8 + 128-tap FIR (SAME) + x8 gain.

Polyphase formulation: with m indexing 64-sample rows of x and n = 8*q' + r in [0, 512),
    y[512*m + n] = sum_{k=0}^{78} T4[k, m] * H4[k, n]
where T4[k, m] = x[64*m + k - 7] (zero-padded) and
    H4[k, 8*q'+r] = 8 * h[(7-r) + 8*(k-q')]  for 0 <= k-q' <= 15, else 0.

Per core (8 cores, batch-parallel): 16 signals (8 batch rows x {real, imag}).
Per signal: load x with halo as [128, 271] (partition p = x[256p-7 : 256p+264]),
PE-transpose four 79-column slices into T4 [79, 512] (columns interleaved m = 4p + c),
then 4 matmuls lhsT=T4[:, 128t:128t+128], rhs=H4 [79, 512] -> PSUM [128, 512],
copy to SBUF, DMA out contiguously (partition i of tile t holds y[65536t + 512i : +512]).
"""

import numpy as np

import concourse.bass as bass
import concourse.tile as tile
from concourse import bacc, mybir
from concourse.bass_utils import run_bass_kernel_spmd

B = 64
N = 32768
FACTOR = 8
NOUT = N * FACTOR  # 262144
N_CORES = 8
ROWS_PER_CORE = B // N_CORES  # 8
SIGS = 2 * ROWS_PER_CORE  # 16 signals per core (real rows then imag rows)
K = 79  # contraction window length
XCOLS = 271  # 256 + 15 halo
TILES = 4  # out tiles per signal, each [128 m-rows, 512 samples]

_F32R = mybir.dt.float32r
_F32 = mybir.dt.float32

_NC_CACHE = {}


def _build_nc():
    nc = bacc.Bacc(
        "TRN2",
        target_bir_lowering=False,
        debug=False,
        enable_asserts=False,
        num_devices=N_CORES,
    )
    x = nc.dram_tensor("x", [SIGS, N], _F32R, kind="ExternalInput")
    h4 = nc.dram_tensor("h4", [K, 512], _F32R, kind="ExternalInput")
    ident = nc.dram_tensor("ident", [128, 128], _F32R, kind="ExternalInput")
    y = nc.dram_tensor("y", [SIGS, NOUT], _F32, kind="ExternalOutput")

    with tile.TileContext(nc) as tc:
        with (
            tc.tile_pool(name="consts", bufs=1) as consts,
            tc.tile_pool(name="xpool", bufs=3) as xpool,
            tc.tile_pool(name="t4pool", bufs=2) as t4pool,
            tc.tile_pool(name="opool", bufs=2) as opool,
            tc.tile_pool(name="pt", bufs=2, space="PSUM") as pt_pool,
            tc.tile_pool(name="po", bufs=4, space="PSUM") as po_pool,
        ):
            h4_sb = consts.tile([K, 512], _F32R)
            nc.sync.dma_start(out=h4_sb, in_=h4.ap())
            ident_sb = consts.tile([128, 128], _F32R)
            nc.sync.dma_start(out=ident_sb, in_=ident.ap())

            for sig in range(SIGS):
                xoff = sig * N
                X = xpool.tile([128, XCOLS], _F32R)
                # partition p holds x[256p - 7 : 256p + 264]; edges zero-padded.
                # Compute-engine partition base must be 0/32/64/96, so zero the
                # [96:128] tail block first and let the main DMA overwrite the
                # valid part of it.
                nc.vector.memset(X[0:1, 0:7].bitcast(_F32), 0.0)
                nc.vector.memset(X[96:128, 263:XCOLS].bitcast(_F32), 0.0)
                nc.sync.dma_start(
                    out=X[0:1, 7:XCOLS],
                    in_=bass.AP(tensor=x, offset=xoff, ap=[[0, 1], [1, 264]]),
                )
                nc.sync.dma_start(
                    out=X[1:127, :],
                    in_=bass.AP(
                        tensor=x, offset=xoff + 249, ap=[[256, 126], [1, XCOLS]]
                    ),
                )
                nc.sync.dma_start(
                    out=X[127:128, 0:263],
                    in_=bass.AP(tensor=x, offset=xoff + 32505, ap=[[0, 1], [1, 263]]),
                )

                # T4[k, 4p + c] = X[p, 64c + k] = x[256p + 64c + k - 7]
                T4 = t4pool.tile([K, 512], _F32R)
                T4i = T4[:, :].rearrange("k (p four) -> k four p", four=4)
                for c in range(4):
                    pt = pt_pool.tile([K, 128], _F32R)
                    nc.tensor.transpose(pt, X[:, 64 * c : 64 * c + K], ident_sb)
                    eng = nc.vector if c % 2 == 0 else nc.scalar
                    if c % 2 == 0:
                        nc.vector.tensor_copy(out=T4i[:, c, :], in_=pt[:, :])
                    else:
                        nc.scalar.copy(out=T4i[:, c, :], in_=pt[:, :])

                out_sb = opool.tile([128, TILES * 512], _F32)
                for t in range(TILES):
                    po = po_pool.tile([128, 512], _F32)
                    nc.tensor.matmul(
                        po,
                        T4[:, 128 * t : 128 * (t + 1)],
                        h4_sb[:, :],
                        start=True,
                        stop=True,
                    )
                    if t % 2 == 0:
                        nc.scalar.copy(out=out_sb[:, 512 * t : 512 * (t + 1)], in_=po)
                    else:
                        nc.vector.tensor_copy(
                            out=out_sb[:, 512 * t : 512 * (t + 1)], in_=po
                        )

                # partition i, free (t, n) -> y[sig, 65536t + 512i + n]
                nc.scalar.dma_start(
                    out=bass.AP(
                        tensor=y,
                        offset=sig * NOUT,
                        ap=[[512, 128], [65536, TILES], [1, 512]],
                    ),
                    in_=out_sb[:, :],
                )

    nc.compile()
    return nc


def _get_nc():
    if "nc" not in _NC_CACHE:
        _NC_CACHE["nc"] = _build_nc()
    return _NC_CACHE["nc"]


def _build_h4(h):
    h4 = np.zeros((K, 512), np.float32)
    qp = np.arange(64)
    for t in range(16):
        for r in range(8):
            h4[qp + t, 8 * qp + r] = FACTOR * h[(7 - r) + 8 * t]
    return h4


def _run(x_real, x_imag, fir_filter, trace=False):
    h4 = _build_h4(np.asarray(fir_filter, np.float32))
    ident = np.eye(128, dtype=np.float32)
    in_maps = []
    for c in range(N_CORES):
        rows = slice(c * ROWS_PER_CORE, (c + 1) * ROWS_PER_CORE)
        shard = np.ascontiguousarray(
            np.concatenate([x_real[rows], x_imag[rows]], axis=0)
        )
        in_maps.append({"x": shard, "h4": h4, "ident": ident})
    nc = _get_nc()
    res = run_bass_kernel_spmd(nc, in_maps, core_ids=list(range(N_CORES)), trace=trace)
    out = np.empty((2, B, NOUT), np.float32)
    for c in range(N_CORES):
        yc = res.results[c]["y"]
        rows = slice(c * ROWS_PER_CORE, (c + 1) * ROWS_PER_CORE)
        out[0, rows] = yc[:ROWS_PER_CORE]
        out[1, rows] = yc[ROWS_PER_CORE:]
    return out, res


def kernel(x_real, x_imag, fir_filter, factor):
    assert int(factor) == FACTOR
    x_real = np.asarray(x_real, np.float32)
    x_imag = np.asarray(x_imag, np.float32)
    assert x_real.shape == (B, N) and x_imag.shape == (B, N)
    out, _ = _run(x_real, x_imag, fir_filter)
    return out
